# revision 1
# baseline (speedup 1.0000x reference)
"""Self-attention (1x1-conv QKV projections + NxN softmax attention + residual)
for x:(4,256,64,64) on 8 TRN2 NeuronCores.

Sharding: core = 2*b + h  ->  batch b in 0..3, query-half h in 0..1.
Each core computes out[b][:, h*2048:(h+1)*2048] (softmax is row-wise over
keys, so splitting query rows is embarrassingly parallel).

Per-core kernel (all matmuls float32r = PE fast-fp32 mode, 1 cycle/row):
  k_sb (32,4096) = Wk@x + bk, q_sb (32,2048) = Wq@x[:,msl] + bq
  v_sb (128,32,256): v^T tiles, v_T[n,c] = sum_c' x[c',n] WvT[c',c]
  energy (transposed, per key-tile pair): e[n,m] = sum_d k[d,n] q[d,m]
    -> (128,1024) PSUM pairs, double-buffered
  p = exp(e / sqrt(32))    (no max subtraction: |e*s| <~ 6, exp is <=2ulp)
  out[c,m] = sum_n v_T[n,c] p[n,m]   (K=128 full-array matmuls, PSUM-accum)
  rowsum[m] = sum_n p[n,m] via ones-lhsT matmuls accumulating on one bank
  final: out = out_raw / rowsum + bv + x_residual

k/q/v projections are interleaved per 512-column x-chunk so the PE starts
as soon as the first input DMA lands instead of waiting for all of x.
"""

import numpy as np

B, C, N = 4, 256, 4096
DK = 32
MH = N // 2          # 2048 query rows per core
NT = N // 128        # 32 key tiles
SBW = 512            # query superblock width
SCALE = 1.0 / float(np.sqrt(DK))

_cache = {}


def _build_nc():
    from contextlib import ExitStack

    import concourse.bacc as bacc
    import concourse.bass as bass
    import concourse.mybir as mybir
    import concourse.tile as tile

    f32 = mybir.dt.float32
    f32r = mybir.dt.float32r
    Exp = mybir.ActivationFunctionType.Exp
    add = mybir.AluOpType.add
    mult = mybir.AluOpType.mult

    nc = bacc.Bacc("TRN2", target_bir_lowering=False, debug=False)

    xf_d = nc.dram_tensor("xf", [C, N], f32r, kind="ExternalInput").ap()
    xq_d = nc.dram_tensor("xq", [C, MH], f32r, kind="ExternalInput").ap()
    wqt_d = nc.dram_tensor("wqt", [C, DK], f32r, kind="ExternalInput").ap()
    wkt_d = nc.dram_tensor("wkt", [C, DK], f32r, kind="ExternalInput").ap()
    wvt_d = nc.dram_tensor("wvt", [C, C], f32r, kind="ExternalInput").ap()
    bq_d = nc.dram_tensor("bq", [DK, 1], f32, kind="ExternalInput").ap()
    bk_d = nc.dram_tensor("bk", [DK, 1], f32, kind="ExternalInput").ap()
    bv_d = nc.dram_tensor("bv", [C, 1], f32, kind="ExternalInput").ap()
    ones_d = nc.dram_tensor("ones32", [128, DK], f32r, kind="ExternalInput").ap()
    out_d = nc.dram_tensor("out", [C, MH], f32, kind="ExternalOutput").ap()

    with tile.TileContext(nc) as tc, ExitStack() as ctx:
        const = ctx.enter_context(tc.tile_pool(name="const", bufs=1))

        # ---- weights / biases to SBUF ----
        wqt_sb = const.tile([128, 2, DK], f32r)
        wkt_sb = const.tile([128, 2, DK], f32r)
        wvt_sb = const.tile([128, 2, C], f32r)

        def split_c(dram_ap, width):
            # (256, width) -> stream (p, a, m) matching a [128, 2, width] tile
            return bass.AP(tensor=dram_ap.tensor, offset=dram_ap.offset,
                           ap=[[width, 128], [128 * width, 2], [1, width]])

        nc.sync.dma_start(out=wqt_sb, in_=split_c(wqt_d, DK))
        nc.sync.dma_start(out=wkt_sb, in_=split_c(wkt_d, DK))

        # ---- inputs: xq first (q unblocks the main loop), then x chunks ----
        x_sb = const.tile([128, 2, N], f32r)
        xq_sb = const.tile([128, 2, MH], f32r)

        def chunk_c(dram_ap, width, j, cw):
            return bass.AP(tensor=dram_ap.tensor, offset=dram_ap.offset + j * cw,
                           ap=[[width, 128], [128 * width, 2], [1, cw]])

        for j in range(4):
            nc.sync.dma_start(out=xq_sb[:, :, j * 512:(j + 1) * 512],
                              in_=chunk_c(xq_d, MH, j, 512))
        bq_sb = const.tile([DK, 1], f32)
        bk_sb = const.tile([DK, 1], f32)
        bv_sb = const.tile([128, 2], f32)
        nc.sync.dma_start(out=bq_sb, in_=bq_d)
        nc.sync.dma_start(out=bk_sb, in_=bk_d)
        for a in range(2):
            nc.sync.dma_start(out=bv_sb[:, a:a + 1], in_=bv_d[a * 128:(a + 1) * 128, :])

        ones = const.tile([128, DK], f32r)
        nc.sync.dma_start(out=ones, in_=ones_d)

        nc.sync.dma_start(out=wvt_sb, in_=split_c(wvt_d, C))
        for j in range(8):
            nc.sync.dma_start(out=x_sb[:, :, j * 512:(j + 1) * 512],
                              in_=chunk_c(xf_d, N, j, 512))

        k_sb = const.tile([DK, N], f32r)
        q_sb = const.tile([DK, MH], f32r)
        v_sb = const.tile([128, NT, C], f32r)

        # ---- projections, interleaved per x-chunk ----
        with tc.tile_pool(name="proj_ps", bufs=2, space="PSUM") as pp:
            for j in range(MH // SBW):
                qp = pp.tile([DK, SBW], f32, name="qp", tag="qp")
                for a in range(2):
                    nc.tensor.matmul(
                        qp, wqt_sb[:, a, :],
                        xq_sb[:, a, j * SBW:(j + 1) * SBW],
                        start=(a == 0), stop=(a == 1))
                nc.vector.tensor_scalar_add(
                    q_sb[:, j * SBW:(j + 1) * SBW], qp, bq_sb)
            for j in range(N // SBW):
                kp = pp.tile([DK, SBW], f32, name="kp", tag="kp")
                for a in range(2):
                    nc.tensor.matmul(
                        kp, wkt_sb[:, a, :],
                        x_sb[:, a, j * SBW:(j + 1) * SBW],
                        start=(a == 0), stop=(a == 1))
                nc.vector.tensor_scalar_add(
                    k_sb[:, j * SBW:(j + 1) * SBW], kp, bk_sb)
                for t in range(4 * j, 4 * j + 4):
                    vp = pp.tile([128, C], f32, name="vp", tag="vp")
                    for a in range(2):
                        nc.tensor.matmul(
                            vp,
                            x_sb[:, a, t * 128:(t + 1) * 128],
                            wvt_sb[:, a, :],
                            start=(a == 0), stop=(a == 1))
                    nc.vector.tensor_copy(out=v_sb[:, t, :], in_=vp)

        # ---- main attention loop: 16 key-tile pairs per query superblock ----
        ep = ctx.enter_context(tc.tile_pool(name="e_ps", bufs=2, space="PSUM"))
        op = ctx.enter_context(tc.tile_pool(name="o_ps", bufs=1, space="PSUM"))
        rp = ctx.enter_context(tc.tile_pool(name="rs_ps", bufs=2, space="PSUM"))
        ppool = ctx.enter_context(tc.tile_pool(name="p_sb", bufs=3))
        misc = ctx.enter_context(tc.tile_pool(name="misc", bufs=2))
        outp = ctx.enter_context(tc.tile_pool(name="outp", bufs=2))

        for sbk in range(MH // SBW):
            msl = slice(sbk * SBW, (sbk + 1) * SBW)
            o_ps = [op.tile([128, SBW], f32, name=f"o_ps{c}", tag=f"o_ps{c}")
                    for c in range(2)]
            rs_ps = rp.tile([DK, SBW], f32)
            # 1-stage software pipeline: emit energy(pr) ahead of PV(pr-1)
            # so the PE FIFO never head-of-line blocks on exp(pr-1).
            pend = None
            for pr in range(NT // 2 + 1):
                if pr < NT // 2:
                    e_pair = ep.tile([128, 2 * SBW], f32,
                                     name="e_pair", tag="e_pair")
                    for i in range(2):
                        t = 2 * pr + i
                        nc.tensor.matmul(
                            e_pair[:, i * SBW:(i + 1) * SBW],
                            k_sb[:, t * 128:(t + 1) * 128],
                            q_sb[:, msl],
                            start=True, stop=True)
                if pend is not None:
                    p_prev, pr_prev = pend
                    for i in range(2):
                        t = 2 * pr_prev + i
                        prhs = p_prev[:, i * SBW:(i + 1) * SBW]
                        for c in range(2):
                            nc.tensor.matmul(
                                o_ps[c],
                                v_sb[:, t, c * 128:(c + 1) * 128],
                                prhs,
                                start=(t == 0), stop=(t == NT - 1))
                        nc.tensor.matmul(
                            rs_ps, ones, prhs,
                            start=(t == 0), stop=(t == NT - 1))
                if pr < NT // 2:
                    p_pair = ppool.tile([128, 2 * SBW], f32r,
                                        name="p_pair", tag="p_pair")
                    nc.scalar.activation(p_pair, e_pair, Exp, scale=SCALE)
                    pend = (p_pair, pr)

            # softmax denominator: partitions 0-31 of rs_ps all hold rowsum
            rec = misc.tile([1, SBW], f32)
            nc.vector.reciprocal(out=rec, in_=rs_ps[0:1, :])
            rec_rep = misc.tile([128, SBW], f32)
            nc.gpsimd.partition_broadcast(rec_rep, rec)

            for c in range(2):
                osb = outp.tile([128, SBW], f32, name=f"osb{c}", tag=f"osb{c}")
                nc.vector.scalar_tensor_tensor(
                    out=osb, in0=o_ps[c], scalar=0.0, in1=rec_rep,
                    op0=add, op1=mult)
                ofin = outp.tile([128, SBW], f32, name=f"ofin{c}", tag=f"ofin{c}")
                nc.vector.scalar_tensor_tensor(
                    out=ofin, in0=osb, scalar=bv_sb[:, c:c + 1],
                    in1=xq_sb[:, c, msl].bitcast(f32), op0=add, op1=add)
                nc.sync.dma_start(out=out_d[c * 128:(c + 1) * 128, msl], in_=ofin)

    nc.compile()
    return nc


def kernel(x, Wq, bq, Wk, bk, Wv, bv):
    from concourse import bass_utils

    x = np.asarray(x, np.float32)
    xf = np.ascontiguousarray(x.reshape(B, C, N))
    wqt = np.ascontiguousarray(np.asarray(Wq, np.float32).T)
    wkt = np.ascontiguousarray(np.asarray(Wk, np.float32).T)
    wvt = np.ascontiguousarray(np.asarray(Wv, np.float32).T)
    bq2 = np.ascontiguousarray(np.asarray(bq, np.float32).reshape(DK, 1))
    bk2 = np.ascontiguousarray(np.asarray(bk, np.float32).reshape(DK, 1))
    bv2 = np.ascontiguousarray(np.asarray(bv, np.float32).reshape(C, 1))
    ones32 = np.ones((128, DK), np.float32)

    if "nc" not in _cache:
        _cache["nc"] = _build_nc()
    nc = _cache["nc"]

    in_maps = []
    for core in range(8):
        b, h = core // 2, core % 2
        in_maps.append({
            "xf": xf[b],
            "xq": np.ascontiguousarray(xf[b][:, h * MH:(h + 1) * MH]),
            "wqt": wqt, "wkt": wkt, "wvt": wvt,
            "bq": bq2, "bk": bk2, "bv": bv2,
            "ones32": ones32,
        })

    res = bass_utils.run_bass_kernel_spmd(nc, in_maps, core_ids=list(range(8)))
    out = np.empty((B, C, N), np.float32)
    for core in range(8):
        b, h = core // 2, core % 2
        out[b][:, h * MH:(h + 1) * MH] = res.results[core]["out"]
    return out.reshape(B, C, 64, 64)



# revision 5
# speedup vs baseline: 1.3699x; 1.3699x over previous
"""Self-attention (1x1-conv QKV projections + NxN softmax attention + residual)
for x:(4,256,64,64) on 8 TRN2 NeuronCores.

Sharding: core = 2*b + h  ->  batch b in 0..3, query-half h in 0..1.
Each core computes out[b][:, h*2048:(h+1)*2048] (softmax is row-wise over
keys, so splitting query rows is embarrassingly parallel).

Per-core kernel, fp8(e4m3)-DoubleRow edition:
  k_sb (32,4096) f32r = Wk8 (x)DR x8 + bk      [DR over the 2 channel halves]
  q_sb (32,2048) f32r = Wq @ xq + bq           [f32r, feeds energy exactly]
  v_sb (128,32,256) fp8: v^T tiles via DR(x8 tile, Wv8), cast PSUM->fp8
  energy (f32r, per key-tile pair): e[n,m] = sum_d k[d,n] q[d,m] -> PSUM f32
  p = exp(e/sqrt(32) - 3) in fp8e4  (shift keeps p<=105<240=fp8 max; the
      e^-3 factor cancels in the softmax normalization)
  PV:  out[c,m] += sum_i v[:,2t+i,ch].T (x) p_pair[:,i,:]   fp8 DoubleRow,
       key-tile pairs are the DR pair dim (layouts line up for free)
  rowsum via ones(x)DR p_pair accumulated in PSUM -> exactly the same
       quantized p as PV, so normalization cancels fp8 bias
  final: out = out_raw / rowsum + bv + x_residual

fp8 notes: every fp8 value lives in [2^-9, 240] where IEEE e4m3 (ml_dtypes,
used for host-side casts) and the HW e4m3 agree bit-for-bit.
"""

import numpy as np

B, C, N = 4, 256, 4096
DK = 32
MH = N // 2          # 2048 query rows per core
NT = N // 128        # 32 key tiles
SBW = 512            # query superblock width
SCALE = 1.0 / float(np.sqrt(DK))
ESHIFT = -3.0        # exp(e*SCALE + ESHIFT): max e*SCALE ~ 7.7 -> p <= ~105

_cache = {}


def _build_nc():
    from contextlib import ExitStack

    import concourse.bacc as bacc
    import concourse.bass as bass
    import concourse.mybir as mybir
    import concourse.tile as tile

    f32 = mybir.dt.float32
    f32r = mybir.dt.float32r
    f8 = mybir.dt.float8e4
    DR = mybir.MatmulPerfMode.DoubleRow
    Exp = mybir.ActivationFunctionType.Exp
    add = mybir.AluOpType.add
    mult = mybir.AluOpType.mult

    nc = bacc.Bacc("TRN2", target_bir_lowering=False, debug=False)

    x8_d = nc.dram_tensor("x8", [C, N], f8, kind="ExternalInput").ap()
    xq_d = nc.dram_tensor("xq", [C, MH], f32r, kind="ExternalInput").ap()
    wqt_d = nc.dram_tensor("wqt", [C, DK], f32r, kind="ExternalInput").ap()
    wkt_d = nc.dram_tensor("wkt8", [C, DK], f8, kind="ExternalInput").ap()
    wvt_d = nc.dram_tensor("wvt8", [C, C], f8, kind="ExternalInput").ap()
    bq_d = nc.dram_tensor("bq", [DK, 1], f32, kind="ExternalInput").ap()
    bk_d = nc.dram_tensor("bk", [DK, 1], f32, kind="ExternalInput").ap()
    bv_d = nc.dram_tensor("bv", [C, 1], f32, kind="ExternalInput").ap()
    ones_d = nc.dram_tensor("ones8", [128, 2 * DK], f8, kind="ExternalInput").ap()
    out_d = nc.dram_tensor("out", [C, MH], f32, kind="ExternalOutput").ap()

    with tile.TileContext(nc) as tc, ExitStack() as ctx:
        const = ctx.enter_context(tc.tile_pool(name="const", bufs=1))

        # ---- weights / biases to SBUF ----
        wqt_sb = const.tile([128, 2, DK], f32r)
        wkt_sb = const.tile([128, 2, DK], f8)
        wvt_sb = const.tile([128, 2, C], f8)

        def split_c(dram_ap, width):
            # (256, width) -> stream (p, a, m) matching a [128, 2, width] tile
            return bass.AP(tensor=dram_ap.tensor, offset=dram_ap.offset,
                           ap=[[width, 128], [128 * width, 2], [1, width]])

        nc.sync.dma_start(out=wqt_sb, in_=split_c(wqt_d, DK))
        nc.sync.dma_start(out=wkt_sb, in_=split_c(wkt_d, DK))
        nc.sync.dma_start(out=wvt_sb, in_=split_c(wvt_d, C))

        bq_sb = const.tile([DK, 1], f32)
        bk_sb = const.tile([DK, 1], f32)
        bv_sb = const.tile([128, 2], f32)
        nc.sync.dma_start(out=bq_sb, in_=bq_d)
        nc.sync.dma_start(out=bk_sb, in_=bk_d)
        for a in range(2):
            nc.sync.dma_start(out=bv_sb[:, a:a + 1], in_=bv_d[a * 128:(a + 1) * 128, :])
        ones = const.tile([128, 2, DK], f8)
        nc.sync.dma_start(out=ones, in_=ones_d)
        eshift_sb = const.tile([128, 1], f32)
        nc.vector.memset(eshift_sb, ESHIFT)

        # ---- inputs: xq chunk0 first (q_sb0 gates the main loop), then x8
        #      chunks (gate k/v proj), then the rest of xq ----
        x8_sb = const.tile([128, 2, N], f8)
        xq_sb = const.tile([128, 2, MH], f32r)

        def chunk_c(dram_ap, width, j, cw):
            return bass.AP(tensor=dram_ap.tensor, offset=dram_ap.offset + j * cw,
                           ap=[[width, 128], [128 * width, 2], [1, cw]])

        nc.sync.dma_start(out=xq_sb[:, :, 0:512], in_=chunk_c(xq_d, MH, 0, 512))
        for j in range(8):
            nc.sync.dma_start(out=x8_sb[:, :, j * 512:(j + 1) * 512],
                              in_=chunk_c(x8_d, N, j, 512))
        for j in range(1, 4):
            nc.sync.dma_start(out=xq_sb[:, :, j * 512:(j + 1) * 512],
                              in_=chunk_c(xq_d, MH, j, 512))

        k_sb = const.tile([DK, N], f32r)
        q_sb = const.tile([DK, MH], f32r)
        v_sb = const.tile([128, NT, C], f8)

        # ---- projections ----
        with tc.tile_pool(name="proj_ps", bufs=2, space="PSUM") as pp:
            # q chunk 0 first: it unblocks superblock 0
            def q_chunk(j):
                qp = pp.tile([DK, SBW], f32, name="qp", tag="qp")
                for a in range(2):
                    nc.tensor.matmul(
                        qp, wqt_sb[:, a, :],
                        xq_sb[:, a, j * SBW:(j + 1) * SBW],
                        start=(a == 0), stop=(a == 1))
                nc.vector.tensor_scalar_add(
                    q_sb[:, j * SBW:(j + 1) * SBW], qp, bq_sb)

            q_chunk(0)
            for j in range(N // SBW):
                kp = pp.tile([DK, SBW], f32, name="kp", tag="kp")
                nc.tensor.matmul(kp, wkt_sb,
                                 x8_sb[:, :, j * SBW:(j + 1) * SBW],
                                 start=True, stop=True, perf_mode=DR)
                nc.vector.tensor_scalar_add(
                    k_sb[:, j * SBW:(j + 1) * SBW], kp, bk_sb)
                for t in range(4 * j, 4 * j + 4):
                    vp = pp.tile([128, C], f32, name="vp", tag="vp")
                    nc.tensor.matmul(vp,
                                     x8_sb[:, :, t * 128:(t + 1) * 128],
                                     wvt_sb, start=True, stop=True,
                                     perf_mode=DR)
                    # GPSIMD cannot read PSUM, so all casts go on DVE
                    nc.vector.tensor_copy(out=v_sb[:, t, :], in_=vp)
            for j in range(1, MH // SBW):
                q_chunk(j)

        # ---- main attention loop: 16 key-tile pairs per query superblock ----
        ep = ctx.enter_context(tc.tile_pool(name="e_ps", bufs=2, space="PSUM"))
        op = ctx.enter_context(tc.tile_pool(name="o_ps", bufs=1, space="PSUM"))
        rp = ctx.enter_context(tc.tile_pool(name="rs_ps", bufs=2, space="PSUM"))
        ppool = ctx.enter_context(tc.tile_pool(name="p_sb", bufs=3))
        misc = ctx.enter_context(tc.tile_pool(name="misc", bufs=2))
        outp = ctx.enter_context(tc.tile_pool(name="outp", bufs=2))

        for sbk in range(MH // SBW):
            msl = slice(sbk * SBW, (sbk + 1) * SBW)
            o_ps = [op.tile([128, SBW], f32, name=f"o_ps{c}", tag=f"o_ps{c}")
                    for c in range(2)]
            rs_ps = rp.tile([DK, SBW], f32)
            # 1-stage software pipeline: emit energy(pr) ahead of PV(pr-1)
            # so the PE FIFO never head-of-line blocks on exp(pr-1).
            pend = None
            for pr in range(NT // 2 + 1):
                if pr < NT // 2:
                    e_pair = ep.tile([128, 2, SBW], f32,
                                     name="e_pair", tag="e_pair")
                    for i in range(2):
                        t = 2 * pr + i
                        nc.tensor.matmul(
                            e_pair[:, i, :],
                            k_sb[:, t * 128:(t + 1) * 128],
                            q_sb[:, msl],
                            start=True, stop=True)
                if pend is not None:
                    p_prev, pr_prev = pend
                    for c in range(2):
                        nc.tensor.matmul(
                            o_ps[c],
                            v_sb[:, 2 * pr_prev:2 * pr_prev + 2,
                                 c * 128:(c + 1) * 128],
                            p_prev,
                            start=(pr_prev == 0), stop=(pr_prev == NT // 2 - 1),
                            perf_mode=DR)
                    nc.tensor.matmul(
                        rs_ps, ones, p_prev,
                        start=(pr_prev == 0), stop=(pr_prev == NT // 2 - 1),
                        perf_mode=DR)
                if pr < NT // 2:
                    p_pair = ppool.tile([128, 2, SBW], f8,
                                        name="p_pair", tag="p_pair")
                    nc.scalar.activation(p_pair, e_pair, Exp,
                                         bias=eshift_sb, scale=SCALE)
                    pend = (p_pair, pr)

            # softmax denominator: partitions 0-31 of rs_ps all hold rowsum
            rec = misc.tile([1, SBW], f32)
            nc.vector.reciprocal(out=rec, in_=rs_ps[0:1, :])
            rec_rep = misc.tile([128, SBW], f32)
            nc.gpsimd.partition_broadcast(rec_rep, rec)

            for c in range(2):
                osb = outp.tile([128, SBW], f32, name=f"osb{c}", tag=f"osb{c}")
                nc.vector.scalar_tensor_tensor(
                    out=osb, in0=o_ps[c], scalar=0.0, in1=rec_rep,
                    op0=add, op1=mult)
                ofin = outp.tile([128, SBW], f32, name=f"ofin{c}", tag=f"ofin{c}")
                nc.vector.scalar_tensor_tensor(
                    out=ofin, in0=osb, scalar=bv_sb[:, c:c + 1],
                    in1=xq_sb[:, c, msl].bitcast(f32), op0=add, op1=add)
                nc.sync.dma_start(out=out_d[c * 128:(c + 1) * 128, msl], in_=ofin)

    nc.compile()
    return nc


def kernel(x, Wq, bq, Wk, bk, Wv, bv):
    import ml_dtypes
    from concourse import bass_utils

    f8 = ml_dtypes.float8_e4m3

    x = np.asarray(x, np.float32)
    xf = np.ascontiguousarray(x.reshape(B, C, N))
    x8 = np.ascontiguousarray(xf.astype(f8))
    wqt = np.ascontiguousarray(np.asarray(Wq, np.float32).T)
    wkt8 = np.ascontiguousarray(np.asarray(Wk, np.float32).T.astype(f8))
    wvt8 = np.ascontiguousarray(np.asarray(Wv, np.float32).T.astype(f8))
    bq2 = np.ascontiguousarray(np.asarray(bq, np.float32).reshape(DK, 1))
    bk2 = np.ascontiguousarray(np.asarray(bk, np.float32).reshape(DK, 1))
    bv2 = np.ascontiguousarray(np.asarray(bv, np.float32).reshape(C, 1))
    ones8 = np.ones((128, 2 * DK), f8)

    if "nc" not in _cache:
        _cache["nc"] = _build_nc()
    nc = _cache["nc"]

    in_maps = []
    for core in range(8):
        b, h = core // 2, core % 2
        in_maps.append({
            "x8": x8[b],
            "xq": np.ascontiguousarray(xf[b][:, h * MH:(h + 1) * MH]),
            "wqt": wqt, "wkt8": wkt8, "wvt8": wvt8,
            "bq": bq2, "bk": bk2, "bv": bv2,
            "ones8": ones8,
        })

    res = bass_utils.run_bass_kernel_spmd(nc, in_maps, core_ids=list(range(8)))
    out = np.empty((B, C, N), np.float32)
    for core in range(8):
        b, h = core // 2, core % 2
        out[b][:, h * MH:(h + 1) * MH] = res.results[core]["out"]
    return out.reshape(B, C, 64, 64)


# revision 18
# speedup vs baseline: 1.5740x; 1.1490x over previous
"""Self-attention (1x1-conv QKV projections + NxN softmax attention + residual)
for x:(4,256,64,64) on 8 TRN2 NeuronCores.

Sharding: core = 2*b + h  ->  batch b in 0..3, query-half h in 0..1.
Each core computes out[b][:, h*2048:(h+1)*2048] (softmax is row-wise over
keys, so splitting query rows is embarrassingly parallel).

Per-core kernel, fp8(e4m3)-DoubleRow edition:
  k_sb (32,4096) f32r = Wk8 (x)DR x8 + bk      [DR over the 2 channel halves]
  q_sb (32,2048) f32r = Wq @ xq + bq           [f32r, feeds energy exactly]
  v_sb (128,32,256) fp8: v^T tiles via DR(x8 tile, Wv8), cast PSUM->fp8
  energy (f32r, per key-tile pair): e[n,m] = sum_d k[d,n] q[d,m] -> PSUM f32
  p = exp(e/sqrt(32) - 3) in fp8e4  (shift keeps p<=105<240=fp8 max; the
      e^-3 factor cancels in the softmax normalization)
  PV:  out[c,m] += sum_i v[:,2t+i,ch].T (x) p_pair[:,i,:]   fp8 DoubleRow,
       key-tile pairs are the DR pair dim (layouts line up for free)
  rowsum via ones(x)DR p_pair accumulated in PSUM -> exactly the same
       quantized p as PV, so normalization cancels fp8 bias
  final: out = out_raw / rowsum + bv + x_residual

Engine balance: Act is the bottleneck (8.4M exps/core ~ 67us busy), so the
projection phase borrows Act (bias adds + early v casts run there while the
exp stream hasn't started), DMAs are ordered so q(chunk0)+k(chunk0) land
first, and the softmax-denominator broadcast is a K=1 ones matmul on PE
instead of a GPSIMD partition_broadcast.

fp8 notes: every fp8 value lives in [2^-9, 240] where IEEE e4m3 (ml_dtypes,
used for host-side casts) and the HW e4m3 agree bit-for-bit.
"""

import numpy as np

B, C, N = 4, 256, 4096
DK = 32
MH = N // 2          # 2048 query rows per core
NT = N // 128        # 32 key tiles
SBW = 512            # query superblock width
SCALE = 1.0 / float(np.sqrt(DK))
ESHIFT = -3.0        # exp(e*SCALE + ESHIFT): max e*SCALE ~ 7.7 -> p <= ~105

_cache = {}


def _build_nc():
    from contextlib import ExitStack

    import concourse.bacc as bacc
    import concourse.bass as bass
    import concourse.mybir as mybir
    import concourse.tile as tile

    f32 = mybir.dt.float32
    f32r = mybir.dt.float32r
    f8 = mybir.dt.float8e4
    DR = mybir.MatmulPerfMode.DoubleRow
    Exp = mybir.ActivationFunctionType.Exp
    add = mybir.AluOpType.add
    mult = mybir.AluOpType.mult

    nc = bacc.Bacc("TRN2", target_bir_lowering=False, debug=False)

    x8_d = nc.dram_tensor("x8", [C, N], f8, kind="ExternalInput").ap()
    xq_d = nc.dram_tensor("xq", [C, MH], f32r, kind="ExternalInput").ap()
    wqt_d = nc.dram_tensor("wqt", [C, DK], f32r, kind="ExternalInput").ap()
    wkt_d = nc.dram_tensor("wkt8", [C, DK], f8, kind="ExternalInput").ap()
    wvt_d = nc.dram_tensor("wvt8", [C, C], f8, kind="ExternalInput").ap()
    bq_d = nc.dram_tensor("bq", [DK, 1], f32, kind="ExternalInput").ap()
    bk_d = nc.dram_tensor("bk", [DK, 1], f32, kind="ExternalInput").ap()
    bv_d = nc.dram_tensor("bv", [C, 1], f32, kind="ExternalInput").ap()
    ones_d = nc.dram_tensor("ones8", [128, 2 * 128], f8, kind="ExternalInput").ap()
    out_d = nc.dram_tensor("out", [C, MH], f32, kind="ExternalOutput").ap()

    with tile.TileContext(nc) as tc, ExitStack() as ctx:
        const = ctx.enter_context(tc.tile_pool(name="const", bufs=1))

        wqt_sb = const.tile([128, 2, DK], f32r)
        wkt_sb = const.tile([128, 2, DK], f8)
        wvt_sb = const.tile([128, 2, C], f8)
        bq_sb = const.tile([DK, 1], f32)
        bk_sb = const.tile([DK, 1], f32)
        bv_sb = const.tile([128, 2], f32)
        ones = const.tile([128, 2, 128], f8)
        x8_sb = const.tile([128, 2, N], f8)
        xq_sb = const.tile([128, 2, MH], f32r)
        eshift_sb = const.tile([128, 1], f32)
        nc.vector.memset(eshift_sb, ESHIFT)

        def split_c(dram_ap, width):
            # (256, width) -> stream (p, a, m) matching a [128, 2, width] tile
            return bass.AP(tensor=dram_ap.tensor, offset=dram_ap.offset,
                           ap=[[width, 128], [128 * width, 2], [1, width]])

        def chunk_c(dram_ap, width, j, cw):
            return bass.AP(tensor=dram_ap.tensor, offset=dram_ap.offset + j * cw,
                           ap=[[width, 128], [128 * width, 2], [1, cw]])

        # DMA order = critical path order: q(chunk0) and k/v(chunk0) gate
        # the first exp; everything else streams behind them.
        nc.sync.dma_start(out=wqt_sb, in_=split_c(wqt_d, DK))
        nc.sync.dma_start(out=wkt_sb, in_=split_c(wkt_d, DK))
        nc.sync.dma_start(out=bq_sb, in_=bq_d)
        nc.sync.dma_start(out=bk_sb, in_=bk_d)
        nc.sync.dma_start(out=xq_sb[:, :, 0:512], in_=chunk_c(xq_d, MH, 0, 512))
        nc.sync.dma_start(out=x8_sb[:, :, 0:512], in_=chunk_c(x8_d, N, 0, 512))
        nc.sync.dma_start(out=wvt_sb, in_=split_c(wvt_d, C))
        nc.sync.dma_start(out=ones, in_=ones_d)
        for j in range(1, 8):
            nc.sync.dma_start(out=x8_sb[:, :, j * 512:(j + 1) * 512],
                              in_=chunk_c(x8_d, N, j, 512))
        nc.sync.dma_start(
            out=xq_sb[:, :, 512:MH],
            in_=bass.AP(tensor=xq_d.tensor, offset=xq_d.offset + 512,
                        ap=[[MH, 128], [128 * MH, 2], [1, MH - 512]]))
        bv_ap = bass.AP(tensor=bv_d.tensor, offset=bv_d.offset,
                        ap=[[1, 128], [128, 2]])
        nc.sync.dma_start(out=bv_sb, in_=bv_ap)

        k_sb = const.tile([DK, N], f32r)
        q_sb = const.tile([DK, MH], f32r)
        v_sb = const.tile([128, NT, C], f8)

        # ---- projections: biases on Act (idle until the exp stream starts),
        #      v casts split Act(early)/DVE(late) ----
        with tc.tile_pool(name="proj_ps", bufs=2, space="PSUM") as pp, \
             tc.tile_pool(name="projv_ps", bufs=3, space="PSUM") as pv:
            def q_chunk(j):
                qp = pp.tile([DK, SBW], f32, name="qp", tag="qp")
                for a in range(2):
                    nc.tensor.matmul(
                        qp, wqt_sb[:, a, :],
                        xq_sb[:, a, j * SBW:(j + 1) * SBW],
                        start=(a == 0), stop=(a == 1))
                nc.scalar.add(q_sb[:, j * SBW:(j + 1) * SBW], qp, bq_sb)

            q_chunk(0)
            for j in range(N // SBW):
                kp = pp.tile([DK, SBW], f32, name="kp", tag="kp")
                nc.tensor.matmul(kp, wkt_sb,
                                 x8_sb[:, :, j * SBW:(j + 1) * SBW],
                                 start=True, stop=True, perf_mode=DR)
                nc.scalar.add(k_sb[:, j * SBW:(j + 1) * SBW], kp, bk_sb)
                for t in range(4 * j, 4 * j + 4):
                    vp = pv.tile([128, C], f32, name="vp", tag="vp")
                    nc.tensor.matmul(vp,
                                     x8_sb[:, :, t * 128:(t + 1) * 128],
                                     wvt_sb, start=True, stop=True,
                                     perf_mode=DR)
                    # GPSIMD cannot read PSUM; split casts Act/DVE
                    if t < 8:
                        nc.scalar.copy(out=v_sb[:, t, :], in_=vp)
                    else:
                        nc.vector.tensor_copy(out=v_sb[:, t, :], in_=vp)
            for j in range(1, MH // SBW):
                q_chunk(j)

        # ---- main attention loop: 16 key-tile pairs per query superblock ----
        ep = ctx.enter_context(tc.tile_pool(name="e_ps", bufs=2, space="PSUM"))
        op = ctx.enter_context(tc.tile_pool(name="o_ps", bufs=1, space="PSUM"))
        rp = ctx.enter_context(tc.tile_pool(name="rs_ps", bufs=1, space="PSUM"))
        ppool = ctx.enter_context(tc.tile_pool(name="p_sb", bufs=3))
        misc = ctx.enter_context(tc.tile_pool(name="misc", bufs=2))
        outp = ctx.enter_context(tc.tile_pool(name="outp", bufs=2))

        for sbk in range(MH // SBW):
            msl = slice(sbk * SBW, (sbk + 1) * SBW)
            o_ps = [op.tile([128, SBW], f32, name=f"o_ps{c}", tag=f"o_ps{c}")
                    for c in range(2)]
            rs_ps = rp.tile([128, SBW], f32)
            # 1-stage software pipeline: emit energy(pr) ahead of PV(pr-1)
            # so the PE FIFO never head-of-line blocks on exp(pr-1).
            pend = None
            for pr in range(NT // 2 + 1):
                if pr < NT // 2:
                    e_pair = ep.tile([128, 2, SBW], f32,
                                     name="e_pair", tag="e_pair")
                    for i in range(2):
                        t = 2 * pr + i
                        nc.tensor.matmul(
                            e_pair[:, i, :],
                            k_sb[:, t * 128:(t + 1) * 128],
                            q_sb[:, msl],
                            start=True, stop=True)
                if pend is not None:
                    p_prev, pr_prev = pend
                    for c in range(2):
                        nc.tensor.matmul(
                            o_ps[c],
                            v_sb[:, 2 * pr_prev:2 * pr_prev + 2,
                                 c * 128:(c + 1) * 128],
                            p_prev,
                            start=(pr_prev == 0), stop=(pr_prev == NT // 2 - 1),
                            perf_mode=DR)
                    nc.tensor.matmul(
                        rs_ps, ones, p_prev,
                        start=(pr_prev == 0), stop=(pr_prev == NT // 2 - 1),
                        perf_mode=DR)
                if pr < NT // 2:
                    p_pair = ppool.tile([128, 2, SBW], f8,
                                        name="p_pair", tag="p_pair")
                    nc.scalar.activation(p_pair, e_pair, Exp,
                                         bias=eshift_sb, scale=SCALE)
                    pend = (p_pair, pr)

            # softmax denominator: the ones matmul put rowsum in ALL 128
            # partitions (M=128 costs the same as M=32), so the reciprocal
            # is partition-parallel and lands in SBUF -- no broadcast step.
            rec_rep = misc.tile([128, SBW], f32)
            nc.vector.reciprocal(out=rec_rep, in_=rs_ps)

            ofin = outp.tile([128, 2, SBW], f32, name="ofin", tag="ofin")
            for c in range(2):
                osb = outp.tile([128, SBW], f32, name=f"osb{c}", tag=f"osb{c}")
                nc.vector.scalar_tensor_tensor(
                    out=osb, in0=o_ps[c], scalar=0.0, in1=rec_rep,
                    op0=add, op1=mult)
                nc.vector.scalar_tensor_tensor(
                    out=ofin[:, c, :], in0=osb, scalar=bv_sb[:, c:c + 1],
                    in1=xq_sb[:, c, msl].bitcast(f32), op0=add, op1=add)
            nc.sync.dma_start(out=chunk_c(out_d, MH, sbk, SBW), in_=ofin)

    nc.compile()
    return nc


def kernel(x, Wq, bq, Wk, bk, Wv, bv):
    import ml_dtypes
    from concourse import bass_utils

    f8 = ml_dtypes.float8_e4m3

    x = np.asarray(x, np.float32)
    xf = np.ascontiguousarray(x.reshape(B, C, N))
    x8 = np.ascontiguousarray(xf.astype(f8))
    wqt = np.ascontiguousarray(np.asarray(Wq, np.float32).T)
    wkt8 = np.ascontiguousarray(np.asarray(Wk, np.float32).T.astype(f8))
    wvt8 = np.ascontiguousarray(np.asarray(Wv, np.float32).T.astype(f8))
    bq2 = np.ascontiguousarray(np.asarray(bq, np.float32).reshape(DK, 1))
    bk2 = np.ascontiguousarray(np.asarray(bk, np.float32).reshape(DK, 1))
    bv2 = np.ascontiguousarray(np.asarray(bv, np.float32).reshape(C, 1))
    ones8 = np.ones((128, 2 * 128), f8)

    if "nc" not in _cache:
        _cache["nc"] = _build_nc()
    nc = _cache["nc"]

    in_maps = []
    for core in range(8):
        b, h = core // 2, core % 2
        in_maps.append({
            "x8": x8[b],
            "xq": np.ascontiguousarray(xf[b][:, h * MH:(h + 1) * MH]),
            "wqt": wqt, "wkt8": wkt8, "wvt8": wvt8,
            "bq": bq2, "bk": bk2, "bv": bv2,
            "ones8": ones8,
        })

    res = bass_utils.run_bass_kernel_spmd(nc, in_maps, core_ids=list(range(8)))
    out = np.empty((B, C, N), np.float32)
    for core in range(8):
        b, h = core // 2, core % 2
        out[b][:, h * MH:(h + 1) * MH] = res.results[core]["out"]
    return out.reshape(B, C, 64, 64)


# revision 19
# speedup vs baseline: 1.6669x; 1.0590x over previous
"""Self-attention (1x1-conv QKV projections + NxN softmax attention + residual)
for x:(4,256,64,64) on 8 TRN2 NeuronCores.

Sharding: core = 2*b + h  ->  batch b in 0..3, query-half h in 0..1.
Each core computes out[b][:, h*2048:(h+1)*2048] (softmax is row-wise over
keys, so splitting query rows is embarrassingly parallel).

Per-core kernel, fp8(e4m3)-DoubleRow edition:
  k_sb (32,4096) f32r = Wk8 (x)DR x8 + bk      [DR over the 2 channel halves]
  q_sb (32,2048) f32r = Wq @ xq + bq           [f32r, feeds energy exactly]
  v_sb (128,32,256) fp8: v^T tiles via DR(x8 tile, Wv8), cast PSUM->fp8
  energy (f32r, per key-tile pair): e[n,m] = sum_d k[d,n] q[d,m] -> PSUM f32
  p = exp(e/sqrt(32) - 3) in fp8e4  (shift keeps p<=105<240=fp8 max; the
      e^-3 factor cancels in the softmax normalization)
  PV:  out[c,m] += sum_i v[:,2t+i,ch].T (x) p_pair[:,i,:]   fp8 DoubleRow,
       key-tile pairs are the DR pair dim (layouts line up for free)
  rowsum via ones(x)DR p_pair accumulated in PSUM -> exactly the same
       quantized p as PV, so normalization cancels fp8 bias
  final: out = out_raw / rowsum + bv + x_residual

Engine balance: Act is the bottleneck (8.4M exps/core ~ 67us busy), so the
projection phase borrows Act (bias adds + early v casts run there while the
exp stream hasn't started), DMAs are ordered so q(chunk0)+k(chunk0) land
first, and the softmax-denominator broadcast is a K=1 ones matmul on PE
instead of a GPSIMD partition_broadcast.

fp8 notes: every fp8 value lives in [2^-9, 240] where IEEE e4m3 (ml_dtypes,
used for host-side casts) and the HW e4m3 agree bit-for-bit.
"""

import numpy as np

B, C, N = 4, 256, 4096
DK = 32
MH = N // 2          # 2048 query rows per core
NT = N // 128        # 32 key tiles
SBW = 512            # query superblock width
SCALE = 1.0 / float(np.sqrt(DK))
ESHIFT = -3.0        # exp(e*SCALE + ESHIFT): max e*SCALE ~ 7.7 -> p <= ~105

_cache = {}


def _build_nc():
    from contextlib import ExitStack

    import concourse.bacc as bacc
    import concourse.bass as bass
    import concourse.mybir as mybir
    import concourse.tile as tile

    f32 = mybir.dt.float32
    f32r = mybir.dt.float32r
    f8 = mybir.dt.float8e4
    DR = mybir.MatmulPerfMode.DoubleRow
    Exp = mybir.ActivationFunctionType.Exp
    add = mybir.AluOpType.add
    mult = mybir.AluOpType.mult

    nc = bacc.Bacc("TRN2", target_bir_lowering=False, debug=False)

    x8_d = nc.dram_tensor("x8", [C, N], f8, kind="ExternalInput").ap()
    xq_d = nc.dram_tensor("xq", [C, MH], f32r, kind="ExternalInput").ap()
    wqt_d = nc.dram_tensor("wqt", [C, DK], f32r, kind="ExternalInput").ap()
    wkt_d = nc.dram_tensor("wkt8", [C, DK], f8, kind="ExternalInput").ap()
    wvt_d = nc.dram_tensor("wvt8", [C, C], f8, kind="ExternalInput").ap()
    bq_d = nc.dram_tensor("bq", [DK, 1], f32, kind="ExternalInput").ap()
    bk_d = nc.dram_tensor("bk", [DK, 1], f32, kind="ExternalInput").ap()
    bv_d = nc.dram_tensor("bv", [C, 1], f32, kind="ExternalInput").ap()
    ones_d = nc.dram_tensor("ones8", [128, 2 * 128], f8, kind="ExternalInput").ap()
    out_d = nc.dram_tensor("out", [C, MH], f32, kind="ExternalOutput").ap()

    with tile.TileContext(nc) as tc, ExitStack() as ctx:
        const = ctx.enter_context(tc.tile_pool(name="const", bufs=1))

        wqt_sb = const.tile([128, 2, DK], f32r)
        wkt_sb = const.tile([128, 2, DK], f8)
        wvt_sb = const.tile([128, 2, C], f8)
        bq_sb = const.tile([DK, 1], f32)
        bk_sb = const.tile([DK, 1], f32)
        bv_sb = const.tile([128, 2], f32)
        ones = const.tile([128, 2, 128], f8)
        x8_sb = const.tile([128, 2, N], f8)
        xq_sb = const.tile([128, 2, MH], f32r)
        eshift_sb = const.tile([128, 1], f32)
        nc.vector.memset(eshift_sb, ESHIFT)

        def split_c(dram_ap, width):
            # (256, width) -> stream (p, a, m) matching a [128, 2, width] tile
            return bass.AP(tensor=dram_ap.tensor, offset=dram_ap.offset,
                           ap=[[width, 128], [128 * width, 2], [1, width]])

        def chunk_c(dram_ap, width, j, cw):
            return bass.AP(tensor=dram_ap.tensor, offset=dram_ap.offset + j * cw,
                           ap=[[width, 128], [128 * width, 2], [1, cw]])

        # DMA order = critical path order: q(chunk0) and k/v(chunk0) gate
        # the first exp; everything else streams behind them.
        nc.sync.dma_start(out=wqt_sb, in_=split_c(wqt_d, DK))
        nc.sync.dma_start(out=wkt_sb, in_=split_c(wkt_d, DK))
        nc.sync.dma_start(out=bq_sb, in_=bq_d)
        nc.sync.dma_start(out=bk_sb, in_=bk_d)
        nc.sync.dma_start(out=xq_sb[:, :, 0:512], in_=chunk_c(xq_d, MH, 0, 512))
        nc.sync.dma_start(out=x8_sb[:, :, 0:512], in_=chunk_c(x8_d, N, 0, 512))
        nc.sync.dma_start(out=wvt_sb, in_=split_c(wvt_d, C))
        nc.sync.dma_start(out=ones, in_=ones_d)
        for j in range(1, 8):
            nc.sync.dma_start(out=x8_sb[:, :, j * 512:(j + 1) * 512],
                              in_=chunk_c(x8_d, N, j, 512))
        nc.sync.dma_start(
            out=xq_sb[:, :, 512:MH],
            in_=bass.AP(tensor=xq_d.tensor, offset=xq_d.offset + 512,
                        ap=[[MH, 128], [128 * MH, 2], [1, MH - 512]]))
        bv_ap = bass.AP(tensor=bv_d.tensor, offset=bv_d.offset,
                        ap=[[1, 128], [128, 2]])
        nc.sync.dma_start(out=bv_sb, in_=bv_ap)

        k_sb = const.tile([DK, N], f32r)
        q_sb = const.tile([DK, MH], f32r)
        v_sb = const.tile([128, NT, C], f8)

        # ---- shared pools (PSUM: e 4 banks, live for the whole kernel) ----
        ep = ctx.enter_context(tc.tile_pool(name="e_ps", bufs=2, space="PSUM"))
        # p pool: sb0's 16 pairs stay alive until its deferred PV burst
        ppool = ctx.enter_context(tc.tile_pool(name="p_sb", bufs=20))
        misc = ctx.enter_context(tc.tile_pool(name="misc", bufs=2))
        outp = ctx.enter_context(tc.tile_pool(name="outp", bufs=2))

        def energy_exp(sbk, pr):
            msl = slice(sbk * SBW, (sbk + 1) * SBW)
            e_pair = ep.tile([128, 2, SBW], f32, name="e_pair", tag="e_pair")
            for i in range(2):
                t = 2 * pr + i
                nc.tensor.matmul(
                    e_pair[:, i, :],
                    k_sb[:, t * 128:(t + 1) * 128],
                    q_sb[:, msl],
                    start=True, stop=True)
            p_pair = ppool.tile([128, 2, SBW], f8, name="p_pair", tag="p_pair")
            nc.scalar.activation(p_pair, e_pair, Exp,
                                 bias=eshift_sb, scale=SCALE)
            return p_pair

        def emit_pv(pr, p_pair, o_ps, rs_ps):
            # rowsum first: its stop-flag matmul gates the reciprocal
            nc.tensor.matmul(
                rs_ps, ones, p_pair,
                start=(pr == 0), stop=(pr == NT // 2 - 1), perf_mode=DR)
            for c in range(2):
                nc.tensor.matmul(
                    o_ps[c],
                    v_sb[:, 2 * pr:2 * pr + 2, c * 128:(c + 1) * 128],
                    p_pair,
                    start=(pr == 0), stop=(pr == NT // 2 - 1), perf_mode=DR)

        # ---- phase A: projections with sb0's energy+exp interleaved so the
        #      Act exp stream starts once q(chunk0)+k(chunk0) exist.  sb0's
        #      PV is deferred to a PE burst after the proj pools close
        #      (PSUM: kqp 2 + vp 2 + e 4 = 8 banks).  Only the two bias adds
        #      gating exp#0 run on Act; everything else goes to DVE. ----
        sb0_pairs = []
        with tc.tile_pool(name="proj_ps", bufs=2, space="PSUM") as pp, \
             tc.tile_pool(name="projv_ps", bufs=2, space="PSUM") as pv:
            def q_chunk(j, on_act):
                qp = pp.tile([DK, SBW], f32, name="kqp", tag="kqp")
                for a in range(2):
                    nc.tensor.matmul(
                        qp, wqt_sb[:, a, :],
                        xq_sb[:, a, j * SBW:(j + 1) * SBW],
                        start=(a == 0), stop=(a == 1))
                if on_act:
                    nc.scalar.add(q_sb[:, j * SBW:(j + 1) * SBW], qp, bq_sb)
                else:
                    nc.vector.tensor_scalar_add(
                        q_sb[:, j * SBW:(j + 1) * SBW], qp, bq_sb)

            q_chunk(0, True)
            for j in range(N // SBW):
                kp = pp.tile([DK, SBW], f32, name="kqp", tag="kqp")
                nc.tensor.matmul(kp, wkt_sb,
                                 x8_sb[:, :, j * SBW:(j + 1) * SBW],
                                 start=True, stop=True, perf_mode=DR)
                if j == 0:
                    nc.scalar.add(k_sb[:, j * SBW:(j + 1) * SBW], kp, bk_sb)
                else:
                    nc.vector.tensor_scalar_add(
                        k_sb[:, j * SBW:(j + 1) * SBW], kp, bk_sb)
                for i in (2 * j, 2 * j + 1):
                    sb0_pairs.append(energy_exp(0, i))
                for t in range(4 * j, 4 * j + 4):
                    vp = pv.tile([128, C], f32, name="vp", tag="vp")
                    nc.tensor.matmul(vp,
                                     x8_sb[:, :, t * 128:(t + 1) * 128],
                                     wvt_sb, start=True, stop=True,
                                     perf_mode=DR)
                    # GPSIMD cannot read PSUM; casts go on DVE (only the
                    # deferred PV burst consumes v -- off the critical path)
                    nc.vector.tensor_copy(out=v_sb[:, t, :], in_=vp)
            for j in range(1, MH // SBW):
                q_chunk(j, False)

        # ---- PV/rowsum pools in the banks freed by the proj pools ----
        op = ctx.enter_context(tc.tile_pool(name="o_ps", bufs=1, space="PSUM"))
        rp = ctx.enter_context(tc.tile_pool(name="rs_ps", bufs=1, space="PSUM"))

        for sbk in range(MH // SBW):
            msl = slice(sbk * SBW, (sbk + 1) * SBW)
            o_ps = [op.tile([128, SBW], f32, name=f"o_ps{c}", tag=f"o_ps{c}")
                    for c in range(2)]
            rs_ps = rp.tile([128, SBW], f32)
            if sbk == 0:
                # deferred PV burst over stored sb0 p_pairs; overlaps the
                # tail of sb0's exp stream and sb1's energy
                for pr in range(NT // 2):
                    emit_pv(pr, sb0_pairs[pr], o_ps, rs_ps)
            else:
                # 1-stage software pipeline: emit energy(pr) ahead of
                # PV(pr-1) so the PE FIFO never blocks on exp(pr-1).
                pend = None
                for pr in range(NT // 2 + 1):
                    p_new = energy_exp(sbk, pr) if pr < NT // 2 else None
                    if pend is not None:
                        emit_pv(pend[1], pend[0], o_ps, rs_ps)
                    pend = (p_new, pr) if p_new is not None else None

            # softmax denominator: the ones matmul put rowsum in ALL 128
            # partitions (M=128 costs the same as M=32), so the reciprocal
            # is partition-parallel and lands in SBUF -- no broadcast step.
            rec_rep = misc.tile([128, SBW], f32)
            nc.vector.reciprocal(out=rec_rep, in_=rs_ps)

            ofin = outp.tile([128, 2, SBW], f32, name="ofin", tag="ofin")
            for c in range(2):
                osb = outp.tile([128, SBW], f32, name=f"osb{c}", tag=f"osb{c}")
                nc.vector.scalar_tensor_tensor(
                    out=osb, in0=o_ps[c], scalar=0.0, in1=rec_rep,
                    op0=add, op1=mult)
                nc.vector.scalar_tensor_tensor(
                    out=ofin[:, c, :], in0=osb, scalar=bv_sb[:, c:c + 1],
                    in1=xq_sb[:, c, msl].bitcast(f32), op0=add, op1=add)
            nc.sync.dma_start(out=chunk_c(out_d, MH, sbk, SBW), in_=ofin)

    nc.compile()
    return nc


def kernel(x, Wq, bq, Wk, bk, Wv, bv):
    import ml_dtypes
    from concourse import bass_utils

    f8 = ml_dtypes.float8_e4m3

    x = np.asarray(x, np.float32)
    xf = np.ascontiguousarray(x.reshape(B, C, N))
    x8 = np.ascontiguousarray(xf.astype(f8))
    wqt = np.ascontiguousarray(np.asarray(Wq, np.float32).T)
    wkt8 = np.ascontiguousarray(np.asarray(Wk, np.float32).T.astype(f8))
    wvt8 = np.ascontiguousarray(np.asarray(Wv, np.float32).T.astype(f8))
    bq2 = np.ascontiguousarray(np.asarray(bq, np.float32).reshape(DK, 1))
    bk2 = np.ascontiguousarray(np.asarray(bk, np.float32).reshape(DK, 1))
    bv2 = np.ascontiguousarray(np.asarray(bv, np.float32).reshape(C, 1))
    ones8 = np.ones((128, 2 * 128), f8)

    if "nc" not in _cache:
        _cache["nc"] = _build_nc()
    nc = _cache["nc"]

    in_maps = []
    for core in range(8):
        b, h = core // 2, core % 2
        in_maps.append({
            "x8": x8[b],
            "xq": np.ascontiguousarray(xf[b][:, h * MH:(h + 1) * MH]),
            "wqt": wqt, "wkt8": wkt8, "wvt8": wvt8,
            "bq": bq2, "bk": bk2, "bv": bv2,
            "ones8": ones8,
        })

    res = bass_utils.run_bass_kernel_spmd(nc, in_maps, core_ids=list(range(8)))
    out = np.empty((B, C, N), np.float32)
    for core in range(8):
        b, h = core // 2, core % 2
        out[b][:, h * MH:(h + 1) * MH] = res.results[core]["out"]
    return out.reshape(B, C, 64, 64)


# revision 21
# speedup vs baseline: 1.7334x; 1.0399x over previous
"""Self-attention (1x1-conv QKV projections + NxN softmax attention + residual)
for x:(4,256,64,64) on 8 TRN2 NeuronCores.

Sharding: core = 2*b + h  ->  batch b in 0..3, query-half h in 0..1.
Each core computes out[b][:, h*2048:(h+1)*2048] (softmax is row-wise over
keys, so splitting query rows is embarrassingly parallel).

Per-core kernel, fp8(e4m3)-DoubleRow edition:
  k_sb (32,4096) f32r = Wk8 (x)DR x8 + bk      [DR over the 2 channel halves]
  q_sb (32,2048) f32r = Wq @ xq + bq           [f32r, feeds energy exactly]
  v_sb (128,32,256) fp8: v^T tiles via DR(x8 tile, Wv8), cast PSUM->fp8
  energy (f32r, per key-tile pair): e[n,m] = sum_d k[d,n] q[d,m] -> PSUM f32
  p = exp(e/sqrt(32) - 3) in fp8e4  (shift keeps p<=105<240=fp8 max; the
      e^-3 factor cancels in the softmax normalization)
  PV:  out[c,m] += sum_i v[:,2t+i,ch].T (x) p_pair[:,i,:]   fp8 DoubleRow,
       key-tile pairs are the DR pair dim (layouts line up for free)
  rowsum via ones(x)DR p_pair accumulated in PSUM -> exactly the same
       quantized p as PV, so normalization cancels fp8 bias
  final: out = out_raw / rowsum + bv + x_residual

Engine balance: Act is the bottleneck (8.4M exps/core ~ 67us busy), so the
projection phase borrows Act (bias adds + early v casts run there while the
exp stream hasn't started), DMAs are ordered so q(chunk0)+k(chunk0) land
first, and the softmax-denominator broadcast is a K=1 ones matmul on PE
instead of a GPSIMD partition_broadcast.

fp8 notes: every fp8 value lives in [2^-9, 240] where IEEE e4m3 (ml_dtypes,
used for host-side casts) and the HW e4m3 agree bit-for-bit.
"""

import numpy as np

B, C, N = 4, 256, 4096
DK = 32
MH = N // 2          # 2048 query rows per core
NT = N // 128        # 32 key tiles
SBW = 512            # query superblock width
SCALE = 1.0 / float(np.sqrt(DK))
ESHIFT = -3.0        # exp(e*SCALE + ESHIFT): max e*SCALE ~ 7.7 -> p <= ~105

_cache = {}


def _build_nc():
    from contextlib import ExitStack

    import concourse.bacc as bacc
    import concourse.bass as bass
    import concourse.mybir as mybir
    import concourse.tile as tile

    f32 = mybir.dt.float32
    f32r = mybir.dt.float32r
    f8 = mybir.dt.float8e4
    DR = mybir.MatmulPerfMode.DoubleRow
    Exp = mybir.ActivationFunctionType.Exp
    add = mybir.AluOpType.add
    mult = mybir.AluOpType.mult

    nc = bacc.Bacc("TRN2", target_bir_lowering=False, debug=False)

    x8_d = nc.dram_tensor("x8", [C, N], f8, kind="ExternalInput").ap()
    xq_d = nc.dram_tensor("xq", [C, MH], f32r, kind="ExternalInput").ap()
    wqt_d = nc.dram_tensor("wqt", [C, DK], f32r, kind="ExternalInput").ap()
    wkt_d = nc.dram_tensor("wkt8", [C, DK], f8, kind="ExternalInput").ap()
    wvt_d = nc.dram_tensor("wvt8", [C, C], f8, kind="ExternalInput").ap()
    bq_d = nc.dram_tensor("bq", [DK, 1], f32, kind="ExternalInput").ap()
    bk_d = nc.dram_tensor("bk", [DK, 1], f32, kind="ExternalInput").ap()
    bv_d = nc.dram_tensor("bv", [C, 1], f32, kind="ExternalInput").ap()
    ones_d = nc.dram_tensor("ones8", [128, 2 * 128], f8, kind="ExternalInput").ap()
    out_d = nc.dram_tensor("out", [C, MH], f32, kind="ExternalOutput").ap()

    with tile.TileContext(nc) as tc, ExitStack() as ctx:
        const = ctx.enter_context(tc.tile_pool(name="const", bufs=1))

        wqt_sb = const.tile([128, 2, DK], f32r)
        wkt_sb = const.tile([128, 2, DK], f8)
        wvt_sb = const.tile([128, 2, C], f8)
        bq_sb = const.tile([DK, 1], f32)
        bk_sb = const.tile([DK, 1], f32)
        bv_sb = const.tile([128, 2], f32)
        ones = const.tile([128, 2, 128], f8)
        x8_sb = const.tile([128, 2, N], f8)
        xq_sb = const.tile([128, 2, MH], f32r)
        eshift_sb = const.tile([128, 1], f32)
        nc.vector.memset(eshift_sb, ESHIFT)

        def split_c(dram_ap, width):
            # (256, width) -> stream (p, a, m) matching a [128, 2, width] tile
            return bass.AP(tensor=dram_ap.tensor, offset=dram_ap.offset,
                           ap=[[width, 128], [128 * width, 2], [1, width]])

        def chunk_c(dram_ap, width, j, cw):
            return bass.AP(tensor=dram_ap.tensor, offset=dram_ap.offset + j * cw,
                           ap=[[width, 128], [128 * width, 2], [1, cw]])

        # DMA order = critical path order: q(chunk0) and k/v(chunk0) gate
        # the first exp; everything else streams behind them.
        nc.sync.dma_start(out=wqt_sb, in_=split_c(wqt_d, DK))
        nc.sync.dma_start(out=wkt_sb, in_=split_c(wkt_d, DK))
        nc.sync.dma_start(out=bq_sb, in_=bq_d)
        nc.sync.dma_start(out=bk_sb, in_=bk_d)
        nc.sync.dma_start(out=xq_sb[:, :, 0:512], in_=chunk_c(xq_d, MH, 0, 512))
        nc.sync.dma_start(out=x8_sb[:, :, 0:512], in_=chunk_c(x8_d, N, 0, 512))
        nc.sync.dma_start(out=wvt_sb, in_=split_c(wvt_d, C))
        nc.sync.dma_start(out=ones, in_=ones_d)
        for j in range(1, 8):
            nc.sync.dma_start(out=x8_sb[:, :, j * 512:(j + 1) * 512],
                              in_=chunk_c(x8_d, N, j, 512))
        nc.sync.dma_start(
            out=xq_sb[:, :, 512:MH],
            in_=bass.AP(tensor=xq_d.tensor, offset=xq_d.offset + 512,
                        ap=[[MH, 128], [128 * MH, 2], [1, MH - 512]]))
        bv_ap = bass.AP(tensor=bv_d.tensor, offset=bv_d.offset,
                        ap=[[1, 128], [128, 2]])
        nc.sync.dma_start(out=bv_sb, in_=bv_ap)

        k_sb = const.tile([DK, N], f32r)
        q_sb = const.tile([DK, MH], f32r)
        v_sb = const.tile([128, NT, C], f8)

        # ---- shared pools (PSUM: e 4 banks, live for the whole kernel) ----
        ep = ctx.enter_context(tc.tile_pool(name="e_ps", bufs=2, space="PSUM"))
        # p pool: sb0's 16 pairs stay alive until its deferred PV burst
        ppool = ctx.enter_context(tc.tile_pool(name="p_sb", bufs=20))
        misc = ctx.enter_context(tc.tile_pool(name="misc", bufs=2))
        outp = ctx.enter_context(tc.tile_pool(name="outp", bufs=2))

        def energy_exp(sbk, pr):
            msl = slice(sbk * SBW, (sbk + 1) * SBW)
            e_pair = ep.tile([128, 2, SBW], f32, name="e_pair", tag="e_pair")
            for i in range(2):
                t = 2 * pr + i
                nc.tensor.matmul(
                    e_pair[:, i, :],
                    k_sb[:, t * 128:(t + 1) * 128],
                    q_sb[:, msl],
                    start=True, stop=True)
            p_pair = ppool.tile([128, 2, SBW], f8, name="p_pair", tag="p_pair")
            nc.scalar.activation(p_pair, e_pair, Exp,
                                 bias=eshift_sb, scale=SCALE)
            return p_pair

        def emit_pv(pr, p_pair, o_ps, rs_ps):
            # rowsum first: its stop-flag matmul gates the reciprocal
            nc.tensor.matmul(
                rs_ps, ones, p_pair,
                start=(pr == 0), stop=(pr == NT // 2 - 1), perf_mode=DR)
            for c in range(2):
                nc.tensor.matmul(
                    o_ps[c],
                    v_sb[:, 2 * pr:2 * pr + 2, c * 128:(c + 1) * 128],
                    p_pair,
                    start=(pr == 0), stop=(pr == NT // 2 - 1), perf_mode=DR)

        # ---- phase A: projections with sb0's energy+exp interleaved so the
        #      Act exp stream starts once q(chunk0)+k(chunk0) exist.  sb0's
        #      PV is deferred to a PE burst after the proj pools close
        #      (PSUM: kqp 2 + vp 2 + e 4 = 8 banks).  Only the two bias adds
        #      gating exp#0 run on Act; everything else goes to DVE. ----
        sb0_pairs = []
        with tc.tile_pool(name="proj_ps", bufs=2, space="PSUM") as pp, \
             tc.tile_pool(name="projv_ps", bufs=2, space="PSUM") as pv:
            def q_chunk(j, on_act):
                qp = pp.tile([DK, SBW], f32, name="kqp", tag="kqp")
                for a in range(2):
                    nc.tensor.matmul(
                        qp, wqt_sb[:, a, :],
                        xq_sb[:, a, j * SBW:(j + 1) * SBW],
                        start=(a == 0), stop=(a == 1))
                if on_act:
                    nc.scalar.add(q_sb[:, j * SBW:(j + 1) * SBW], qp, bq_sb)
                else:
                    nc.vector.tensor_scalar_add(
                        q_sb[:, j * SBW:(j + 1) * SBW], qp, bq_sb)

            q_chunk(0, True)
            for j in range(N // SBW):
                kp = pp.tile([DK, SBW], f32, name="kqp", tag="kqp")
                nc.tensor.matmul(kp, wkt_sb,
                                 x8_sb[:, :, j * SBW:(j + 1) * SBW],
                                 start=True, stop=True, perf_mode=DR)
                if j == 0:
                    nc.scalar.add(k_sb[:, j * SBW:(j + 1) * SBW], kp, bk_sb)
                else:
                    nc.vector.tensor_scalar_add(
                        k_sb[:, j * SBW:(j + 1) * SBW], kp, bk_sb)
                for i in (2 * j, 2 * j + 1):
                    sb0_pairs.append(energy_exp(0, i))
            for j in range(1, MH // SBW):
                q_chunk(j, False)
            # v projections last: their DVE casts would otherwise delay the
            # k/q bias adds that gate the exp stream; only the deferred PV
            # burst consumes v, so this is off the critical path.
            for t in range(NT):
                vp = pv.tile([128, C], f32, name="vp", tag="vp")
                nc.tensor.matmul(vp,
                                 x8_sb[:, :, t * 128:(t + 1) * 128],
                                 wvt_sb, start=True, stop=True,
                                 perf_mode=DR)
                # GPSIMD cannot read PSUM; casts go on DVE
                nc.vector.tensor_copy(out=v_sb[:, t, :], in_=vp)

        # ---- PV/rowsum pools in the banks freed by the proj pools ----
        op = ctx.enter_context(tc.tile_pool(name="o_ps", bufs=1, space="PSUM"))
        rp = ctx.enter_context(tc.tile_pool(name="rs_ps", bufs=1, space="PSUM"))

        for sbk in range(MH // SBW):
            msl = slice(sbk * SBW, (sbk + 1) * SBW)
            o_ps = [op.tile([128, SBW], f32, name=f"o_ps{c}", tag=f"o_ps{c}")
                    for c in range(2)]
            rs_ps = rp.tile([128, SBW], f32)
            if sbk == 0:
                # deferred PV burst over stored sb0 p_pairs; overlaps the
                # tail of sb0's exp stream and sb1's energy
                for pr in range(NT // 2):
                    emit_pv(pr, sb0_pairs[pr], o_ps, rs_ps)
            else:
                # 1-stage software pipeline: emit energy(pr) ahead of
                # PV(pr-1) so the PE FIFO never blocks on exp(pr-1).
                pend = None
                for pr in range(NT // 2 + 1):
                    p_new = energy_exp(sbk, pr) if pr < NT // 2 else None
                    if pend is not None:
                        emit_pv(pend[1], pend[0], o_ps, rs_ps)
                    pend = (p_new, pr) if p_new is not None else None

            # softmax denominator: the ones matmul put rowsum in ALL 128
            # partitions (M=128 costs the same as M=32), so the reciprocal
            # is partition-parallel and lands in SBUF -- no broadcast step.
            rec_rep = misc.tile([128, SBW], f32)
            nc.vector.reciprocal(out=rec_rep, in_=rs_ps)

            last = sbk == MH // SBW - 1
            ofin = outp.tile([128, 2, SBW], f32, name="ofin", tag="ofin")
            for c in range(2):
                osb = outp.tile([128, SBW], f32, name=f"osb{c}", tag=f"osb{c}")
                nc.vector.scalar_tensor_tensor(
                    out=osb, in0=o_ps[c], scalar=0.0, in1=rec_rep,
                    op0=add, op1=mult)
                nc.vector.scalar_tensor_tensor(
                    out=ofin[:, c, :], in0=osb, scalar=bv_sb[:, c:c + 1],
                    in1=xq_sb[:, c, msl].bitcast(f32), op0=add, op1=add)
                if last:
                    # tail: per-half DMA so c0's store overlaps c1's math
                    nc.sync.dma_start(
                        out=out_d[c * 128:(c + 1) * 128, msl],
                        in_=ofin[:, c, :])
            if not last:
                nc.sync.dma_start(out=chunk_c(out_d, MH, sbk, SBW), in_=ofin)

    nc.compile()
    return nc


def kernel(x, Wq, bq, Wk, bk, Wv, bv):
    import ml_dtypes
    from concourse import bass_utils

    f8 = ml_dtypes.float8_e4m3

    x = np.asarray(x, np.float32)
    xf = np.ascontiguousarray(x.reshape(B, C, N))
    x8 = np.ascontiguousarray(xf.astype(f8))
    wqt = np.ascontiguousarray(np.asarray(Wq, np.float32).T)
    wkt8 = np.ascontiguousarray(np.asarray(Wk, np.float32).T.astype(f8))
    wvt8 = np.ascontiguousarray(np.asarray(Wv, np.float32).T.astype(f8))
    bq2 = np.ascontiguousarray(np.asarray(bq, np.float32).reshape(DK, 1))
    bk2 = np.ascontiguousarray(np.asarray(bk, np.float32).reshape(DK, 1))
    bv2 = np.ascontiguousarray(np.asarray(bv, np.float32).reshape(C, 1))
    ones8 = np.ones((128, 2 * 128), f8)

    if "nc" not in _cache:
        _cache["nc"] = _build_nc()
    nc = _cache["nc"]

    in_maps = []
    for core in range(8):
        b, h = core // 2, core % 2
        in_maps.append({
            "x8": x8[b],
            "xq": np.ascontiguousarray(xf[b][:, h * MH:(h + 1) * MH]),
            "wqt": wqt, "wkt8": wkt8, "wvt8": wvt8,
            "bq": bq2, "bk": bk2, "bv": bv2,
            "ones8": ones8,
        })

    res = bass_utils.run_bass_kernel_spmd(nc, in_maps, core_ids=list(range(8)))
    out = np.empty((B, C, N), np.float32)
    for core in range(8):
        b, h = core // 2, core % 2
        out[b][:, h * MH:(h + 1) * MH] = res.results[core]["out"]
    return out.reshape(B, C, 64, 64)


# revision 23
# speedup vs baseline: 1.8000x; 1.0384x over previous
"""Self-attention (1x1-conv QKV projections + NxN softmax attention + residual)
for x:(4,256,64,64) on 8 TRN2 NeuronCores.

Sharding: core = 2*b + h  ->  batch b in 0..3, query-half h in 0..1.
Each core computes out[b][:, h*2048:(h+1)*2048] (softmax is row-wise over
keys, so splitting query rows is embarrassingly parallel).

Per-core kernel, fp8(e4m3)-DoubleRow edition:
  k_sb (32,4096) f32r = Wk8 (x)DR x8 + bk      [DR over the 2 channel halves]
  q_sb (32,2048) f32r = Wq @ xq + bq           [f32r, feeds energy exactly]
  v_sb (128,32,256) fp8: v^T tiles via DR(x8 tile, Wv8), cast PSUM->fp8
  energy (f32r, per key-tile pair): e[n,m] = sum_d k[d,n] q[d,m] -> PSUM f32
  p = exp(e/sqrt(32) - 3) in fp8e4  (shift keeps p<=105<240=fp8 max; the
      e^-3 factor cancels in the softmax normalization)
  PV:  out[c,m] += sum_i v[:,2t+i,ch].T (x) p_pair[:,i,:]   fp8 DoubleRow,
       key-tile pairs are the DR pair dim (layouts line up for free)
  rowsum via ones(x)DR p_pair accumulated in PSUM -> exactly the same
       quantized p as PV, so normalization cancels fp8 bias
  final: out = out_raw / rowsum + bv + x_residual

Engine balance: Act is the bottleneck (8.4M exps/core ~ 67us busy), so the
projection phase borrows Act (bias adds + early v casts run there while the
exp stream hasn't started), DMAs are ordered so q(chunk0)+k(chunk0) land
first, and the softmax-denominator broadcast is a K=1 ones matmul on PE
instead of a GPSIMD partition_broadcast.

fp8 notes: every fp8 value lives in [2^-9, 240] where IEEE e4m3 (ml_dtypes,
used for host-side casts) and the HW e4m3 agree bit-for-bit.
"""

import numpy as np

B, C, N = 4, 256, 4096
DK = 32
MH = N // 2          # 2048 query rows per core
NT = N // 128        # 32 key tiles
SBW = 512            # query superblock width
SCALE = 1.0 / float(np.sqrt(DK))
ESHIFT = -3.0        # exp(e*SCALE + ESHIFT): max e*SCALE ~ 7.7 -> p <= ~105

_cache = {}


def _build_nc():
    from contextlib import ExitStack

    import concourse.bacc as bacc
    import concourse.bass as bass
    import concourse.mybir as mybir
    import concourse.tile as tile

    f32 = mybir.dt.float32
    f32r = mybir.dt.float32r
    f8 = mybir.dt.float8e4
    DR = mybir.MatmulPerfMode.DoubleRow
    Exp = mybir.ActivationFunctionType.Exp
    add = mybir.AluOpType.add
    mult = mybir.AluOpType.mult

    nc = bacc.Bacc("TRN2", target_bir_lowering=False, debug=False)

    x8_d = nc.dram_tensor("x8", [C, N], f8, kind="ExternalInput").ap()
    xq_d = nc.dram_tensor("xq", [C, MH], f32r, kind="ExternalInput").ap()
    wqt_d = nc.dram_tensor("wqt", [C, DK], f32r, kind="ExternalInput").ap()
    wkt_d = nc.dram_tensor("wkt8", [C, DK], f8, kind="ExternalInput").ap()
    wvt_d = nc.dram_tensor("wvt8", [C, C], f8, kind="ExternalInput").ap()
    bq_d = nc.dram_tensor("bq", [DK, 1], f32, kind="ExternalInput").ap()
    bk_d = nc.dram_tensor("bk", [DK, 1], f32, kind="ExternalInput").ap()
    bv_d = nc.dram_tensor("bv", [C, 1], f32, kind="ExternalInput").ap()
    ones_d = nc.dram_tensor("ones8", [128, 2 * 128], f8, kind="ExternalInput").ap()
    out_d = nc.dram_tensor("out", [C, MH], f32, kind="ExternalOutput").ap()

    with tile.TileContext(nc) as tc, ExitStack() as ctx:
        const = ctx.enter_context(tc.tile_pool(name="const", bufs=1))

        wqt_sb = const.tile([128, 2, DK], f32r)
        wkt_sb = const.tile([128, 2, DK], f8)
        wvt_sb = const.tile([128, 2, C], f8)
        bq_sb = const.tile([DK, 1], f32)
        bk_sb = const.tile([DK, 1], f32)
        bv_sb = const.tile([128, 2], f32)
        ones = const.tile([128, 2, 128], f8)
        x8_sb = const.tile([128, 2, N], f8)
        xq_sb = const.tile([128, 2, MH], f32r)
        eshift_sb = const.tile([128, 1], f32)
        nc.vector.memset(eshift_sb, ESHIFT)

        def split_c(dram_ap, width):
            # (256, width) -> stream (p, a, m) matching a [128, 2, width] tile
            return bass.AP(tensor=dram_ap.tensor, offset=dram_ap.offset,
                           ap=[[width, 128], [128 * width, 2], [1, width]])

        def chunk_c(dram_ap, width, j, cw):
            return bass.AP(tensor=dram_ap.tensor, offset=dram_ap.offset + j * cw,
                           ap=[[width, 128], [128 * width, 2], [1, cw]])

        # DMA order = critical path order: q(chunk0) and k/v(chunk0) gate
        # the first exp; everything else streams behind them.
        nc.sync.dma_start(out=wqt_sb, in_=split_c(wqt_d, DK))
        nc.sync.dma_start(out=wkt_sb, in_=split_c(wkt_d, DK))
        nc.sync.dma_start(out=bq_sb, in_=bq_d)
        nc.sync.dma_start(out=bk_sb, in_=bk_d)
        nc.sync.dma_start(out=xq_sb[:, :, 0:512], in_=chunk_c(xq_d, MH, 0, 512))
        nc.sync.dma_start(out=x8_sb[:, :, 0:512], in_=chunk_c(x8_d, N, 0, 512))
        nc.sync.dma_start(out=wvt_sb, in_=split_c(wvt_d, C))
        nc.sync.dma_start(out=ones, in_=ones_d)
        for j in range(1, 8):
            nc.sync.dma_start(out=x8_sb[:, :, j * 512:(j + 1) * 512],
                              in_=chunk_c(x8_d, N, j, 512))
        nc.sync.dma_start(
            out=xq_sb[:, :, 512:MH],
            in_=bass.AP(tensor=xq_d.tensor, offset=xq_d.offset + 512,
                        ap=[[MH, 128], [128 * MH, 2], [1, MH - 512]]))
        bv_ap = bass.AP(tensor=bv_d.tensor, offset=bv_d.offset,
                        ap=[[1, 128], [128, 2]])
        nc.sync.dma_start(out=bv_sb, in_=bv_ap)

        k_sb = const.tile([DK, N], f32r)
        q_sb = const.tile([DK, MH], f32r)
        v_sb = const.tile([128, NT, C], f8)

        # ---- shared pools (PSUM: e 4 banks, live for the whole kernel) ----
        ep = ctx.enter_context(tc.tile_pool(name="e_ps", bufs=2, space="PSUM"))
        # p pool: sb0's 16 pairs stay alive until its deferred PV burst
        ppool = ctx.enter_context(tc.tile_pool(name="p_sb", bufs=20))
        misc = ctx.enter_context(tc.tile_pool(name="misc", bufs=2))
        outp = ctx.enter_context(tc.tile_pool(name="outp", bufs=2))

        def energy_exp(sbk, pr):
            msl = slice(sbk * SBW, (sbk + 1) * SBW)
            e_pair = ep.tile([128, 2, SBW], f32, name="e_pair", tag="e_pair")
            for i in range(2):
                t = 2 * pr + i
                nc.tensor.matmul(
                    e_pair[:, i, :],
                    k_sb[:, t * 128:(t + 1) * 128],
                    q_sb[:, msl],
                    start=True, stop=True)
            p_pair = ppool.tile([128, 2, SBW], f8, name="p_pair", tag="p_pair")
            nc.scalar.activation(p_pair, e_pair, Exp,
                                 bias=eshift_sb, scale=SCALE)
            return p_pair

        def emit_pv(pr, p_pair, o_ps, rs_ps):
            # rowsum first: its stop-flag matmul gates the reciprocal
            nc.tensor.matmul(
                rs_ps, ones, p_pair,
                start=(pr == 0), stop=(pr == NT // 2 - 1), perf_mode=DR)
            for c in range(2):
                nc.tensor.matmul(
                    o_ps[c],
                    v_sb[:, 2 * pr:2 * pr + 2, c * 128:(c + 1) * 128],
                    p_pair,
                    start=(pr == 0), stop=(pr == NT // 2 - 1), perf_mode=DR)

        # ---- phase A: projections with sb0's energy+exp interleaved so the
        #      Act exp stream starts once q(chunk0)+k(chunk0) exist.  sb0's
        #      PV is deferred to a PE burst after the proj pools close
        #      (PSUM: kqp 2 + vp 2 + e 4 = 8 banks).  Only the two bias adds
        #      gating exp#0 run on Act; everything else goes to DVE. ----
        sb0_pairs = []
        with tc.tile_pool(name="proj_ps", bufs=2, space="PSUM") as pp, \
             tc.tile_pool(name="projv_ps", bufs=2, space="PSUM") as pv:
            def q_chunk(j, on_act):
                qp = pp.tile([DK, SBW], f32, name="kqp", tag="kqp")
                for a in range(2):
                    nc.tensor.matmul(
                        qp, wqt_sb[:, a, :],
                        xq_sb[:, a, j * SBW:(j + 1) * SBW],
                        start=(a == 0), stop=(a == 1))
                if on_act:
                    nc.scalar.add(q_sb[:, j * SBW:(j + 1) * SBW], qp, bq_sb)
                else:
                    nc.vector.tensor_scalar_add(
                        q_sb[:, j * SBW:(j + 1) * SBW], qp, bq_sb)

            q_chunk(0, True)
            for j in range(N // SBW):
                kp = pp.tile([DK, SBW], f32, name="kqp", tag="kqp")
                nc.tensor.matmul(kp, wkt_sb,
                                 x8_sb[:, :, j * SBW:(j + 1) * SBW],
                                 start=True, stop=True, perf_mode=DR)
                if j == 0:
                    nc.scalar.add(k_sb[:, j * SBW:(j + 1) * SBW], kp, bk_sb)
                else:
                    nc.vector.tensor_scalar_add(
                        k_sb[:, j * SBW:(j + 1) * SBW], kp, bk_sb)
                for i in (2 * j, 2 * j + 1):
                    sb0_pairs.append(energy_exp(0, i))
            for j in range(1, MH // SBW):
                q_chunk(j, False)
            # v projections last: their DVE casts would otherwise delay the
            # k/q bias adds that gate the exp stream; only the deferred PV
            # burst consumes v, so this is off the critical path.
            for t in range(NT):
                vp = pv.tile([128, C], f32, name="vp", tag="vp")
                nc.tensor.matmul(vp,
                                 x8_sb[:, :, t * 128:(t + 1) * 128],
                                 wvt_sb, start=True, stop=True,
                                 perf_mode=DR)
                # GPSIMD cannot read PSUM; casts go on DVE
                nc.vector.tensor_copy(out=v_sb[:, t, :], in_=vp)

        # ---- PV/rowsum pools in the banks freed by the proj pools ----
        op = ctx.enter_context(tc.tile_pool(name="o_ps", bufs=1, space="PSUM"))
        rp = ctx.enter_context(tc.tile_pool(name="rs_ps", bufs=1, space="PSUM"))

        def final_chain(sbk, o_ps, rs_ps, last):
            msl = slice(sbk * SBW, (sbk + 1) * SBW)
            # softmax denominator: the ones matmul put rowsum in ALL 128
            # partitions (M=128 costs the same as M=32), so the reciprocal
            # is partition-parallel and lands in SBUF -- no broadcast step.
            rec_rep = misc.tile([128, SBW], f32)
            nc.vector.reciprocal(out=rec_rep, in_=rs_ps)
            ofin = outp.tile([128, 2, SBW], f32, name="ofin", tag="ofin")
            for c in range(2):
                osb = outp.tile([128, SBW], f32, name=f"osb{c}", tag=f"osb{c}")
                nc.vector.scalar_tensor_tensor(
                    out=osb, in0=o_ps[c], scalar=0.0, in1=rec_rep,
                    op0=add, op1=mult)
                nc.vector.scalar_tensor_tensor(
                    out=ofin[:, c, :], in0=osb, scalar=bv_sb[:, c:c + 1],
                    in1=xq_sb[:, c, msl].bitcast(f32), op0=add, op1=add)
                if last:
                    # tail: per-half DMA so c0's store overlaps c1's math
                    nc.sync.dma_start(
                        out=out_d[c * 128:(c + 1) * 128, msl],
                        in_=ofin[:, c, :])
            if not last:
                nc.sync.dma_start(out=chunk_c(out_d, MH, sbk, SBW), in_=ofin)

        # Uniform defer-by-one pipeline: superblock k's PV/rowsum matmuls
        # ride inside superblock k+1's energy/exp stream (PE has ~290ns of
        # slack per pair under the Act exp pace), so the Act engine never
        # waits for a PV burst at a superblock boundary.  Only one o/rs
        # PSUM set is alive at a time: e 4 + o 2 + rs 1 = 7 banks.
        prev = (sb0_pairs,
                [op.tile([128, SBW], f32, name=f"o_ps{c}", tag=f"o_ps{c}")
                 for c in range(2)],
                rp.tile([128, SBW], f32, name="rs", tag="rs"), 0)
        for sbk in range(1, MH // SBW):
            pairs_k = []
            for pr in range(NT // 2):
                pairs_k.append(energy_exp(sbk, pr))
                emit_pv(pr, prev[0][pr], prev[1], prev[2])
            final_chain(prev[3], prev[1], prev[2], last=False)
            prev = (pairs_k,
                    [op.tile([128, SBW], f32, name=f"o_ps{c}", tag=f"o_ps{c}")
                     for c in range(2)],
                    rp.tile([128, SBW], f32, name="rs", tag="rs"), sbk)
        for pr in range(NT // 2):
            emit_pv(pr, prev[0][pr], prev[1], prev[2])
        final_chain(prev[3], prev[1], prev[2], last=True)

    nc.compile()
    return nc


def kernel(x, Wq, bq, Wk, bk, Wv, bv):
    import ml_dtypes
    from concourse import bass_utils

    f8 = ml_dtypes.float8_e4m3

    x = np.asarray(x, np.float32)
    xf = np.ascontiguousarray(x.reshape(B, C, N))
    x8 = np.ascontiguousarray(xf.astype(f8))
    wqt = np.ascontiguousarray(np.asarray(Wq, np.float32).T)
    wkt8 = np.ascontiguousarray(np.asarray(Wk, np.float32).T.astype(f8))
    wvt8 = np.ascontiguousarray(np.asarray(Wv, np.float32).T.astype(f8))
    bq2 = np.ascontiguousarray(np.asarray(bq, np.float32).reshape(DK, 1))
    bk2 = np.ascontiguousarray(np.asarray(bk, np.float32).reshape(DK, 1))
    bv2 = np.ascontiguousarray(np.asarray(bv, np.float32).reshape(C, 1))
    ones8 = np.ones((128, 2 * 128), f8)

    if "nc" not in _cache:
        _cache["nc"] = _build_nc()
    nc = _cache["nc"]

    in_maps = []
    for core in range(8):
        b, h = core // 2, core % 2
        in_maps.append({
            "x8": x8[b],
            "xq": np.ascontiguousarray(xf[b][:, h * MH:(h + 1) * MH]),
            "wqt": wqt, "wkt8": wkt8, "wvt8": wvt8,
            "bq": bq2, "bk": bk2, "bv": bv2,
            "ones8": ones8,
        })

    res = bass_utils.run_bass_kernel_spmd(nc, in_maps, core_ids=list(range(8)))
    out = np.empty((B, C, N), np.float32)
    for core in range(8):
        b, h = core // 2, core % 2
        out[b][:, h * MH:(h + 1) * MH] = res.results[core]["out"]
    return out.reshape(B, C, 64, 64)
